# revision 1
# baseline (speedup 1.0000x reference)
"""Trainium2 Bass kernel for nn_DualModalHyperGraph (dual-modal hypergraph conv).

Self-contained: builds one SPMD Bass/Tile program for 8 NeuronCores, shards
inputs on the host, runs via run_bass_kernel_spmd, reassembles the output.

Math (equivalent to the reference):
  sim_m = cols-normalized(mean_B feat_m) gram matrix    (per modality m)
  M_mk[n, r] = 1 iff r in top-(k+1) of row n            (two k per modality)
  Sigma_m = sum_k M_mk^T M_mk / (k+1)^2                 ([2048, 2048])
  A = D^-1/2 (blkdiag(Sigma_1, Sigma_2) + 1/4 [[I,I],[I,I]]) D^-1/2
  x1 = relu(A (x @ W1^T)); x2 = relu(A (x1 @ W2^T))

Sharding: core c = 4*m + s handles modality m, 512-row/col slab s. The
runtime supports only AllReduce/ReduceScatter collectives and no
registers/dynamic APs, so all per-core divergence is encoded in
host-provided flag tensors; mask exchange is an AllReduce of a
zero-padded stacked buffer, and the final layer is computed as partial
contributions summed by a ReduceScatter.
"""

import numpy as np

import concourse.bass as bass
import concourse.bacc as bacc
import concourse.mybir as mybir
import concourse.tile as tile
from concourse.bass_utils import run_bass_kernel_spmd
from concourse.masks import make_identity

P = 128
B = 4
N = 2048          # nodes per modality
NN = 2 * N
C = 64
F = 128
NT = 16           # 128-row tiles per modality
GT = 32           # global row tiles
MINVAL = -3.0e38
FP8 = mybir.dt.float8e4
F32 = mybir.dt.float32
F32R = mybir.dt.float32r
AL = mybir.AluOpType
AF = mybir.ActivationFunctionType

_CACHED_NC = None
LAST_EXEC_TIME_NS = None
LAST_RESULTS = None


def build_nc(f32r=False, drow=True):
    nc = bacc.Bacc("TRN2", target_bir_lowering=False, debug=False, num_devices=8)

    xT = nc.dram_tensor("xT", [C, B, NN], F32, kind="ExternalInput")
    xTm = nc.dram_tensor("xTm", [C, B, N], F32, kind="ExternalInput")
    xTmy = nc.dram_tensor("xTmy", [C, B, 512], F32, kind="ExternalInput")
    w1t = nc.dram_tensor("w1t", [C, F], F32, kind="ExternalInput")
    w2t = nc.dram_tensor("w2t", [F, F], F32, kind="ExternalInput")
    slotmask = nc.dram_tensor("slotmask", [P, 24], F32, kind="ExternalInput")
    cconst = nc.dram_tensor("cconst", [P, 8], F32, kind="ExternalInput")
    plflags = nc.dram_tensor("plflags", [P, 8], F32, kind="ExternalInput")
    qisel_in = nc.dram_tensor("qisel", [GT, P, 512], FP8, kind="ExternalInput")
    out_z = nc.dram_tensor("out_z", [B, 512, F], F32, kind="ExternalOutput")

    arin_m = nc.dram_tensor("arin_m", [2, GT, P, N], FP8)
    arout_m = nc.dram_tensor("arout_m", [2, GT, P, N], FP8, addr_space="Shared")
    cs_d = nc.dram_tensor("cs_d", [2, 2, N], F32)
    nsq_d = nc.dram_tensor("nsq_d", [1, N], F32)
    ninv_d = nc.dram_tensor("ninv_d", [1, N], F32)
    d_d = nc.dram_tensor("d_d", [2, N], F32)
    d2_d = nc.dram_tensor("d2_d", [2, N], F32)
    rsin = nc.dram_tensor("rsin", [8, 4, P, B * F], F32)
    rsout = nc.dram_tensor("rsout", [4, P, B * F], F32)

    ALLW = [list(range(8))]

    with tile.TileContext(nc) as tc:
      with tc.tile_pool(name="persist", bufs=1) as pp:
        cc = pp.tile([P, 8], F32)
        pf = pp.tile([P, 8], F32)
        sm = pp.tile([P, 24], F32)
        w1s = pp.tile([C, F], F32)
        w2s = pp.tile([F, F], F32)
        nc.sync.dma_start(cc[:], cconst[:])
        nc.sync.dma_start(pf[:], plflags[:])
        nc.sync.dma_start(sm[:], slotmask[:])
        nc.sync.dma_start(w1s[:], w1t[:])
        nc.sync.dma_start(w2s[:], w2t[:])
        ca0, ca1 = cc[:, 0:1], cc[:, 1:2]
        cb0, cb1 = cc[:, 2:3], cc[:, 3:4]
        f0_11, f1_11 = cc[0:1, 4:5], cc[0:1, 5:6]

        id128 = pp.tile([P, P], F32)
        make_identity(nc, id128[:])
        id32 = pp.tile([32, 32], F32)
        make_identity(nc, id32[:])
        ones8 = pp.tile([P, 1], FP8)
        nc.vector.memset(ones8[:], 1.0)
        idfl = pp.tile([P, 8, P], FP8)
        for k in range(8):
            nc.vector.tensor_scalar(idfl[:, k, :], id128[:], pf[:, k:k + 1], None, AL.mult)

        d_np = pp.tile([P, GT], F32)
        d2_np = pp.tile([P, GT], F32)
        d_own = pp.tile([P, 4], F32)
        d2_own = pp.tile([P, 4], F32)

        # ================= P1 + P2 + P3 =================
        with tc.tile_pool(name="psA", bufs=2, space="PSUM") as psA, \
             tc.tile_pool(name="topk", bufs=1) as tkp, \
             tc.tile_pool(name="tk2", bufs=3) as tk2:

            with tc.tile_pool(name="simprep", bufs=1) as sp:
                xTm_s = sp.tile([C, B, N], F32, tag="xTm_s")
                xTmy_s = sp.tile([C, B, 512], F32, tag="xTmy_s")
                nc.sync.dma_start(xTm_s[:], xTm[:])
                nc.sync.dma_start(xTmy_s[:], xTmy[:])

                fmy = tkp.tile([C, 512], F32)
                nc.vector.tensor_tensor(fmy[:], xTmy_s[:, 0], xTmy_s[:, 1], AL.add)
                nc.vector.tensor_tensor(fmy[:], fmy[:], xTmy_s[:, 2], AL.add)
                nc.vector.tensor_tensor(fmy[:], fmy[:], xTmy_s[:, 3], AL.add)
                nc.vector.tensor_scalar_mul(fmy[:], fmy[:], 0.25)

                fm = sp.tile([C, N], F32)
                nc.vector.tensor_tensor(fm[:], xTm_s[:, 0], xTm_s[:, 1], AL.add)
                nc.vector.tensor_tensor(fm[:], fm[:], xTm_s[:, 2], AL.add)
                nc.vector.tensor_tensor(fm[:], fm[:], xTm_s[:, 3], AL.add)
                nc.vector.tensor_scalar_mul(fm[:], fm[:], 0.25)

                fsq = sp.tile([C, N], F32, tag="xTmy_s")
                nc.vector.tensor_tensor(fsq[:], fm[:], fm[:], AL.mult)
                onesC = sp.tile([C, 1], F32)
                nc.vector.memset(onesC[:], 1.0)
                nsq_sb = sp.tile([1, N], F32)
                for ch in range(4):
                    ps = psA.tile([1, 512], F32, tag="ps1")
                    nc.tensor.matmul(ps[:], lhsT=onesC[:], rhs=fsq[:, ch * 512:(ch + 1) * 512],
                                     start=True, stop=True)
                    nc.scalar.activation(nsq_sb[:, ch * 512:(ch + 1) * 512], ps[:], AF.Copy)
                nc.sync.dma_start(nsq_d[:], nsq_sb[:])
                nperm = sp.tile([P, 16], F32)
                nc.sync.dma_start(nperm[:], nsq_d[:].rearrange("a (p j) -> p (a j)", p=P))
                nc.vector.tensor_scalar_max(nperm[:], nperm[:], 1e-24)
                nc.vector.reciprocal(nperm[:], nperm[:])
                nc.scalar.activation(nperm[:], nperm[:], AF.Sqrt)
                nc.sync.dma_start(ninv_d[:].rearrange("a (p j) -> p (a j)", p=P), nperm[:])
                ninv_rep = sp.tile([C, N], F32, tag="xTm_s")
                nc.sync.dma_start(ninv_rep[:], ninv_d[0:1, :].to_broadcast([C, N]))
                fhat = tkp.tile([C, N], F32)
                nc.vector.tensor_tensor(fhat[:], fm[:], ninv_rep[:], AL.mult)

            # ---- P2: sim rows, topk masks ----
            mA = tkp.tile([P, 4, N], FP8)
            mB = tkp.tile([P, 4, N], FP8)
            scr = tkp.tile([P, 8], F32)
            inv8 = tkp.tile([P, 8], F32)
            for t in range(4):
                sim_sb = tk2.tile([P, N], F32, tag="simsb")
                for ch in range(4):
                    ps = psA.tile([P, 512], F32, tag="ps512")
                    nc.tensor.matmul(ps[:], lhsT=fmy[:, t * P:(t + 1) * P],
                                     rhs=fhat[:, ch * 512:(ch + 1) * 512],
                                     start=True, stop=True)
                    nc.scalar.activation(sim_sb[:, ch * 512:(ch + 1) * 512], ps[:], AF.Copy)
                work = tk2.tile([P, N], F32, tag="work")
                src = sim_sb
                for r in range(3):
                    nc.vector.max(out=scr[:], in_=src[:])
                    nc.vector.tensor_tensor(scr[:], scr[:], sm[:, r * 8:(r + 1) * 8], AL.mult)
                    nc.vector.tensor_scalar(inv8[:], sm[:, r * 8:(r + 1) * 8],
                                            -MINVAL, MINVAL, AL.mult, AL.add)
                    nc.vector.tensor_tensor(scr[:], scr[:], inv8[:], AL.add)
                    nc.vector.match_replace(out=work[:], in_to_replace=scr[:],
                                            in_values=src[:], imm_value=MINVAL)
                    src = work
                    if r == 0:
                        nc.vector.tensor_tensor(mA[:, t, :], work[:], sim_sb[:], AL.not_equal)
                nc.vector.tensor_tensor(mB[:, t, :], work[:], sim_sb[:], AL.not_equal)

            # ---- placement into the mask-AllReduce input (flag-scaled) ----
            for qi, mq in enumerate((mA, mB)):
                mq2 = mq[:].rearrange("p t n -> p (t n)")
                for s in range(8):
                    sc = tk2.tile([P, 4 * N], FP8, tag=f"plc{s % 3}")
                    if s % 3 == 0:
                        nc.scalar.activation(sc[:], mq2, AF.Copy, scale=pf[:, s:s + 1])
                    elif s % 3 == 1:
                        nc.vector.tensor_scalar(sc[:], mq2, pf[:, s:s + 1], None, AL.mult)
                    else:
                        nc.scalar.activation(sc[:], mq2, AF.Copy, scale=pf[:, s:s + 1])
                    for t in range(4):
                        nc.sync.dma_start(arin_m[qi, s * 4 + t], sc[:, t * N:(t + 1) * N])

            # ---- single collective for masks ----
            nc.gpsimd.collective_compute("AllReduce", AL.add, replica_groups=ALLW,
                                         ins=[arin_m[:]], outs=[arout_m[:]])

        # ================= P4: S-build directly into slab =================
        with tc.tile_pool(name="slabp", bufs=1) as slp:
            slab = slp.tile([P, GT, 512], F32)
            with tc.tile_pool(name="spool", bufs=1) as spl, \
                 tc.tile_pool(name="psS", bufs=2, space="PSUM") as psS:
                for qi in range(2):
                    mfq = [spl.tile([P, 8, N], FP8, tag=f"maskfull{i}", name=f"mfq{i}") for i in range(4)]
                    for i in range(4):
                        nc.sync.dma_start(mfq[i][:],
                                          arout_m[qi, 8 * i:8 * (i + 1)].rearrange("g p n -> p g n"))
                    csq = spl.tile([1, 2, N], F32, tag=f"csq{qi}")
                    for mm in range(2):
                        for ch in range(4):
                            psc = psS.tile([1, 512], F32, tag="psc")
                            for gg in range(NT):
                                g_ = mm * NT + gg
                                nc.tensor.matmul(psc[:], lhsT=ones8[:],
                                                 rhs=mfq[g_ // 8][:, g_ % 8, ch * 512:(ch + 1) * 512],
                                                 start=(gg == 0), stop=(gg == NT - 1))
                            nc.scalar.activation(csq[:, mm, ch * 512:(ch + 1) * 512], psc[:], AF.Copy)
                    nc.sync.dma_start(cs_d[qi:qi + 1].rearrange("a q n -> a (q n)"),
                                      csq[:].rearrange("a q n -> a (q n)"))
                    stg = spl.tile([P, GT, 512], FP8, tag="stage")
                    for g in range(GT):
                        mfl = g // NT
                        psg = psS.tile([P, 512], F32, tag="psg")
                        for j in range(4):
                            nc.tensor.matmul(psg[:], lhsT=idfl[:, 4 * mfl + j, :],
                                             rhs=mfq[g // 8][:, g % 8, j * 512:(j + 1) * 512],
                                             start=(j == 0), stop=(j == 3))
                        nc.scalar.activation(stg[:, g, :], psg[:], AF.Copy)
                    w_top = ca0 if qi == 0 else cb0
                    w_bot = ca1 if qi == 0 else cb1
                    for mt in range(NT):
                        pss = psS.tile([P, 512], F32, tag="pss")
                        if drow:
                            for g2 in range(GT // 2):
                                i4, o4 = (2 * g2) // 8, (2 * g2) % 8
                                nc.tensor.matmul(pss[:], lhsT=mfq[i4][:, o4:o4 + 2, mt * P:(mt + 1) * P],
                                                 rhs=stg[:, 2 * g2:2 * g2 + 2, :],
                                                 start=(g2 == 0), stop=(g2 == GT // 2 - 1),
                                                 perf_mode=mybir.MatmulPerfMode.DoubleRow)
                        else:
                            for g in range(GT):
                                nc.tensor.matmul(pss[:], lhsT=mfq[g // 8][:, g % 8, mt * P:(mt + 1) * P],
                                                 rhs=stg[:, g, :], start=(g == 0), stop=(g == GT - 1))
                        if qi == 0:
                            nc.scalar.activation(slab[:, mt, :], pss[:], AF.Copy, scale=w_top)
                            nc.vector.tensor_scalar(slab[:, NT + mt, :], pss[:], w_bot, None, AL.mult)
                        else:
                            tmp = spl.tile([P, 512], F32, tag="cbk")
                            nc.scalar.activation(tmp[:], pss[:], AF.Copy, scale=w_top)
                            nc.vector.tensor_tensor(slab[:, mt, :], slab[:, mt, :], tmp[:], AL.add)
                            tmp2 = spl.tile([P, 512], F32, tag="cbk2")
                            nc.scalar.activation(tmp2[:], pss[:], AF.Copy, scale=w_bot)
                            nc.vector.tensor_tensor(slab[:, NT + mt, :], slab[:, NT + mt, :],
                                                    tmp2[:], AL.add)

                # ---- degree vectors from colsums ----
                for m in range(2):
                    ap_ = spl.tile([P, 16], F32, tag="dva")
                    bp_ = spl.tile([P, 16], F32, tag="dvb")
                    nc.sync.dma_start(ap_[:], cs_d[0, m:m + 1, :].rearrange("a (p j) -> p (a j)", p=P))
                    nc.sync.dma_start(bp_[:], cs_d[1, m:m + 1, :].rearrange("a (p j) -> p (a j)", p=P))
                    nc.vector.tensor_tensor(ap_[:], ap_[:], bp_[:], AL.add)
                    nc.vector.tensor_scalar_add(ap_[:], ap_[:], 1.0)
                    nc.vector.reciprocal(ap_[:], ap_[:])
                    nc.sync.dma_start(d2_d[m:m + 1, :].rearrange("a (p j) -> p (a j)", p=P), ap_[:])
                    nc.scalar.activation(ap_[:], ap_[:], AF.Sqrt)
                    nc.sync.dma_start(d_d[m:m + 1, :].rearrange("a (p j) -> p (a j)", p=P), ap_[:])
                for dst, srcd in ((d_np, d_d), (d2_np, d2_d)):
                    tr_in = spl.tile([32, P], F32, tag="trin")
                    nc.sync.dma_start(tr_in[:], srcd[:].rearrange("m (r c) -> (m r) c", r=16))
                    pst = psS.tile([P, 32], F32, tag="pst")
                    nc.tensor.transpose(pst[:], tr_in[:], id32[:])
                    nc.scalar.activation(dst[:], pst[:], AF.Copy)
                dsel = spl.tile([P, 4], F32, tag="dsel")
                for dst, srcT in ((d_own, d_np), (d2_own, d2_np)):
                    nc.vector.memset(dst[:], 0.0)
                    for g in range(8):
                        nc.vector.tensor_scalar(dsel[:], srcT[:, g * 4:(g + 1) * 4],
                                                pf[:, g:g + 1], None, AL.mult)
                        nc.vector.tensor_tensor(dst[:], dst[:], dsel[:], AL.add)

            # ---- P5: + quarter-identity J blocks ----
            with tc.tile_pool(name="qsp", bufs=1) as qsp:
                qs = qsp.tile([P, GT, 512], FP8)
                nc.sync.dma_start(qs[:], qisel_in[:].rearrange("g p n -> p g n"))
                s2 = slab[:].rearrange("p g n -> p (g n)")
                q2 = qs[:].rearrange("p g n -> p (g n)")
                nc.vector.tensor_tensor(s2[:, 0:NT * 512], s2[:, 0:NT * 512],
                                        q2[:, 0:NT * 512], AL.add)
                nc.vector.tensor_tensor(s2[:, NT * 512:], s2[:, NT * 512:],
                                        q2[:, NT * 512:], AL.add)

            # ================= P6: FM1 + AGG1 =================
            z1T = slp.tile([P, B, 512], F32, tag="z1T")
            with tc.tile_pool(name="fm1", bufs=1) as fmp, \
                 tc.tile_pool(name="xgp", bufs=3) as xgp, \
                 tc.tile_pool(name="psF", bufs=4, space="PSUM") as psF:
                u1 = fmp.tile([P, GT, B, F], F32)
                for g in range(GT):
                    xg = xgp.tile([C, B, P], F32, tag="xg")
                    nc.sync.dma_start(xg[:], xT[:, :, g * P:(g + 1) * P])
                    for b in range(B):
                        psy = psF.tile([P, F], F32, tag="psy")
                        nc.tensor.matmul(psy[:], lhsT=xg[:, b, :], rhs=w1s[:],
                                         start=True, stop=True)
                        nc.scalar.activation(u1[:, g, b, :], psy[:], AF.Copy)
                for g in range(GT):
                    u1g = u1[:, g].rearrange("p b f -> p (b f)")
                    nc.vector.tensor_scalar(u1g, u1g, d_np[:, g:g + 1], None, AL.mult)
                for b in range(B):
                    psz = psF.tile([P, 512], F32, tag="psz")
                    for g in range(GT):
                        if f32r:
                            nc.tensor.matmul(psz[:], lhsT=u1[:, g, b, :].bitcast(F32R),
                                             rhs=slab[:, g, :].bitcast(F32R),
                                             start=(g == 0), stop=(g == GT - 1))
                        else:
                            nc.tensor.matmul(psz[:], lhsT=u1[:, g, b, :], rhs=slab[:, g, :],
                                             start=(g == 0), stop=(g == GT - 1))
                    nc.scalar.activation(z1T[:, b, :], psz[:], AF.Relu)

            # ================= P8: slab^T, FM2, AGG2, RS, out =================
            with tc.tile_pool(name="slabTp", bufs=1) as sTp, \
                 tc.tile_pool(name="psT", bufs=2, space="PSUM") as psT:
                slabT = sTp.tile([P, 4, NN], F32)
                for g in range(GT):
                    for mt in range(4):
                        pstr = psT.tile([P, P], F32, tag="pstr")
                        nc.tensor.transpose(pstr[:], slab[:, g, mt * P:(mt + 1) * P], id128[:])
                        nc.scalar.activation(slabT[:, mt, g * P:(g + 1) * P], pstr[:], AF.Copy)
                u2 = sTp.tile([P, 4, B, F], F32, tag="u2")
                for mt in range(4):
                    for b in range(B):
                        psy2 = psT.tile([P, F], F32, tag="psy2")
                        nc.tensor.matmul(psy2[:], lhsT=z1T[:, b, mt * P:(mt + 1) * P],
                                         rhs=w2s[:], start=True, stop=True)
                        nc.scalar.activation(u2[:, mt, b, :], psy2[:], AF.Copy,
                                             scale=d2_own[:, mt:mt + 1])
                z2sb = sTp.tile([P, B * F], F32, tag="z2sb")
                for g in range(GT):
                    psz2 = psT.tile([P, B * F], F32, tag="psz2")
                    for kt in range(4):
                        if f32r:
                            nc.tensor.matmul(psz2[:], lhsT=slabT[:, kt, g * P:(g + 1) * P].bitcast(F32R),
                                             rhs=u2[:, kt].bitcast(F32R),
                                             start=(kt == 0), stop=(kt == 3))
                        else:
                            nc.tensor.matmul(psz2[:], lhsT=slabT[:, kt, g * P:(g + 1) * P],
                                             rhs=u2[:, kt], start=(kt == 0), stop=(kt == 3))
                    nc.scalar.activation(z2sb[:], psz2[:], AF.Copy)
                    nc.sync.dma_start(rsin[g // 4, g % 4], z2sb[:])

                nc.gpsimd.collective_compute("ReduceScatter", AL.add, replica_groups=ALLW,
                                             ins=[rsin[:]], outs=[rsout[:]])
                zf = sTp.tile([P, 4, B, F], F32, tag="zf")
                nc.sync.dma_start(zf[:], rsout[:].rearrange("t p (b f) -> p t b f", b=B))
                outsb = sTp.tile([P, 4, B, F], F32, tag="outsb")
                for mt in range(4):
                    nc.scalar.activation(outsb[:, mt], zf[:, mt], AF.Relu,
                                         scale=d_own[:, mt:mt + 1])
                for mt in range(4):
                    nc.sync.dma_start(
                        out_z[:, mt * P:(mt + 1) * P, :].rearrange("b p f -> p b f"),
                        outsb[:, mt])

    nc.compile()
    return nc


def _fp8(x):
    return x.astype(mybir.dt.np(FP8))


def _make_inputs(feat_mod1, feat_mod2, W1, W2):
    f1 = np.ascontiguousarray(np.asarray(feat_mod1), np.float32)
    f2 = np.ascontiguousarray(np.asarray(feat_mod2), np.float32)
    xT1 = np.ascontiguousarray(f1.transpose(2, 0, 1))
    xT2 = np.ascontiguousarray(f2.transpose(2, 0, 1))
    xT = np.ascontiguousarray(np.concatenate([xT1, xT2], axis=2))
    w1t = np.ascontiguousarray(np.asarray(W1, np.float32).T)
    w2t = np.ascontiguousarray(np.asarray(W2, np.float32).T)

    KS = {0: (7, 19), 1: (5, 13)}  # k+1 per modality
    in_maps = []
    for c in range(8):
        m, s = c // 4, c % 4
        xTm = xT1 if m == 0 else xT2
        xTmy = np.ascontiguousarray(xTm[:, :, s * 512:(s + 1) * 512])
        kA, kB = KS[m]
        slotm = np.zeros((P, 24), np.float32)
        slotm[:, 0:kA] = 1.0
        slotm[:, 8:16] = 1.0
        rem = kB - kA - 8
        if rem > 0:
            slotm[:, 16:16 + rem] = 1.0
        f0 = 1.0 if m == 0 else 0.0
        f1v = 1.0 - f0
        ccv = np.zeros((P, 8), np.float32)
        ccv[:, 0] = f0 / (kA * kA)   # ca0
        ccv[:, 1] = f1v / (kA * kA)  # ca1
        ccv[:, 2] = f0 / (kB * kB)   # cb0
        ccv[:, 3] = f1v / (kB * kB)  # cb1
        ccv[:, 4] = f0
        ccv[:, 5] = f1v
        plf = np.zeros((P, 8), np.float32)
        plf[:, c] = 1.0
        qis = np.zeros((GT, P, 512), np.float32)
        jj = np.arange(512)
        qis[s * 4 + jj // P, jj % P, jj] = 0.25
        qis[16 + s * 4 + jj // P, jj % P, jj] = 0.25
        in_maps.append({
            "xT": xT, "xTm": xTm, "xTmy": xTmy, "w1t": w1t, "w2t": w2t,
            "slotmask": slotm, "cconst": ccv, "plflags": plf, "qisel": _fp8(qis),
        })
    return in_maps


def kernel(feat_mod1, feat_mod2, W1, W2):
    global _CACHED_NC, LAST_EXEC_TIME_NS, LAST_RESULTS
    if _CACHED_NC is None:
        _CACHED_NC = build_nc()
    in_maps = _make_inputs(feat_mod1, feat_mod2, W1, W2)
    res = run_bass_kernel_spmd(_CACHED_NC, in_maps, list(range(8)))
    LAST_RESULTS = res
    LAST_EXEC_TIME_NS = getattr(res, "exec_time_ns", None)
    outs = [res.results[c]["out_z"] for c in range(8)]
    out1 = np.concatenate(outs[0:4], axis=1)
    out2 = np.concatenate(outs[4:8], axis=1)
    return out1, out2



# revision 7
# speedup vs baseline: 2.0296x; 2.0296x over previous
"""Trainium2 Bass kernel for nn_DualModalHyperGraph (dual-modal hypergraph conv).

Self-contained: builds one SPMD Bass/Tile program for 8 NeuronCores, shards
inputs on the host, runs via run_bass_kernel_spmd, reassembles the output.

Math (equivalent to the reference):
  sim_m = col-normalized(mean_B feat_m) gram matrix     (per modality m)
  M_mk[n, r] = 1 iff r in top-(k+1) of row n            (two k per modality)
  Sigma_m = sum_k M_mk^T M_mk / (k+1)^2                 ([2048, 2048])
  S = blkdiag(Sigma_1, Sigma_2) + 1/4 [[I,I],[I,I]]
  A = D^-1/2 S D^-1/2,  D = diag(rowsum of H)
  x1 = relu(A (x @ W1^T)); x2 = relu(A (x1 @ W2^T))

Distribution (core c = 4*m + s: modality m, 512-row slab s):
  - Each core computes sim rows + top-k masks for its own 512 rows only.
  - Partial Sigma_m (contraction over own rows) is computed locally from
    scaled fp16 masks and summed via a per-modality ReduceScatter that also
    carries partial column-sum (degree) vectors; each core receives
    Sigma_m[own 512 rows, all 2048 modality cols] (slabT) plus its degrees.
  - Both hgconv layers are computed as partial contributions over own rows
    (y += S[own, :]^T u[own]) summed by 8-core ReduceScatters; the J = 1/4
    inter-modality block is applied via host-gated 0.25-identity matmuls.
  - SPMD divergence is data-driven only (host flag/diagonal tensors).
"""

import numpy as np

import concourse.bass as bass
import concourse.bacc as bacc
import concourse.mybir as mybir
import concourse.tile as tile
from concourse.bass_utils import run_bass_kernel_spmd
from concourse.masks import make_identity

P = 128
B = 4
N = 2048          # nodes per modality
C = 64
F = 128
NT = 16           # 128-row tiles per modality
GT = 32           # global row tiles
BF = B * F        # 512
MINVAL = -3.0e38
F16 = mybir.dt.float16
F32 = mybir.dt.float32
AL = mybir.AluOpType
AF = mybir.ActivationFunctionType

SIGD = 4 * P * N          # per-dest Sigma elems in the RS-Sigma buffer
CSD = 2 * 512             # per-dest colsum elems
DD = SIGD + CSD

DEBUG = False
_CACHED_NC = None
LAST_EXEC_TIME_NS = None
LAST_RESULTS = None


def build_nc():
    nc = bacc.Bacc("TRN2", target_bir_lowering=False, debug=False, num_devices=8)

    xTm = nc.dram_tensor("xTm", [C, B, N], F32, kind="ExternalInput")
    xTmy = nc.dram_tensor("xTmy", [C, B, 512], F32, kind="ExternalInput")
    w1t = nc.dram_tensor("w1t", [C, F], F32, kind="ExternalInput")
    w2t = nc.dram_tensor("w2t", [F, F], F16, kind="ExternalInput")
    slotmask = nc.dram_tensor("slotmask", [P, 24], F32, kind="ExternalInput")
    cconst = nc.dram_tensor("cconst", [P, 8], F32, kind="ExternalInput")
    jdiag = nc.dram_tensor("jdiag", [P, GT * P], F16, kind="ExternalInput")
    out_z = nc.dram_tensor("out_z", [B, 512, F], F32, kind="ExternalOutput")
    if DEBUG:
        dbg_mk = nc.dram_tensor("dbg_mk", [P, 8 * N], F16, kind="ExternalOutput")
        dbg_slabT = nc.dram_tensor("dbg_slabT", [P, 4 * N], F16, kind="ExternalOutput")
        dbg_cs = nc.dram_tensor("dbg_cs", [P, 8], F32, kind="ExternalOutput")
        dbg_d = nc.dram_tensor("dbg_d", [P, 8], F32, kind="ExternalOutput")
        dbg_u1d = nc.dram_tensor("dbg_u1d", [P, 4 * BF], F16, kind="ExternalOutput")
        dbg_y1 = nc.dram_tensor("dbg_y1", [P, 4 * BF], F16, kind="ExternalOutput")
        dbg_rsin1 = nc.dram_tensor("dbg_rsin1", [GT, P * BF], F16, kind="ExternalOutput")

    ninv_d = nc.dram_tensor("ninv_d", [1, N], F32)
    csflat_d = nc.dram_tensor("csflat_d", [1, 2 * N], F16)
    rsinS = nc.dram_tensor("rsinS", [4, DD], F16)
    rsoutS = nc.dram_tensor("rsoutS", [1, DD], F16)
    rsin1 = nc.dram_tensor("rsin1", [8, 4, P, BF], F16)
    rsout1 = nc.dram_tensor("rsout1", [4, P, BF], F16)
    rsin2 = nc.dram_tensor("rsin2", [8, 4, P, BF], F16)
    rsout2 = nc.dram_tensor("rsout2", [4, P, BF], F16)

    ALLW = [list(range(8))]
    MODW = [[0, 1, 2, 3], [4, 5, 6, 7]]

    # dram views
    sig_in = rsinS[:, 0:SIGD].rearrange("d (t p n) -> d t p n", t=4, p=P)
    cs_in = rsinS[:, SIGD:DD]                       # [4, 1024]
    sig_out = rsoutS[:, 0:SIGD].rearrange("a (t p n) -> (a t) p n", t=4, p=P)
    cs_out = rsoutS[:, SIGD:DD]                     # [1, 1024]

    with tile.TileContext(nc) as tc:
      with tc.tile_pool(name="persist", bufs=1) as pp:
        cc = pp.tile([P, 8], F32)
        sm = pp.tile([P, 24], F32)
        jdg = pp.tile([P, GT, P], F16)
        w1s = pp.tile([C, F], F32)
        w2s = pp.tile([F, F], F16)
        nc.sync.dma_start(cc[:], cconst[:])
        nc.sync.dma_start(sm[:], slotmask[:])
        nc.sync.dma_start(jdg[:], jdiag[:].rearrange("p (g q) -> p g q", g=GT))
        nc.sync.dma_start(w1s[:], w1t[:])
        nc.sync.dma_start(w2s[:], w2t[:])
        invkA, invkB = cc[:, 0:1], cc[:, 1:2]
        kA1, kB1 = cc[:, 2:3], cc[:, 3:4]
        f0a, f1a = cc[:, 4:5], cc[:, 5:6]

        id128h = pp.tile([P, P], F16)
        make_identity(nc, id128h[:])

        # fp16 top-k masks for own 4 row tiles, both k-levels (scaled later)
        mk = [pp.tile([P, 4, N], F16, name=f"mk{q}") for q in range(2)]
        # degrees at own rows
        d_own = pp.tile([P, 4], F32)
        d2_own = pp.tile([P, 4], F32)
        # slabT = Sigma_m[own 512 rows, all N cols] (fp16, post-RS)
        slabT = pp.tile([P, 4, N], F16)
        # layer activations
        u1 = pp.tile([P, 4, BF], F16)      # x@W1 at own rows (unscaled)
        u1d = pp.tile([P, 4, BF], F16)     # d * u1
        u1g = [pp.tile([P, 4, BF], F16, name=f"u1g{h}") for h in range(2)]
        u2d = pp.tile([P, 4, BF], F16)
        u2g = [pp.tile([P, 4, BF], F16, name=f"u2g{h}") for h in range(2)]

        # ================= mean features, fhat, sim prep =================
        with tc.tile_pool(name="prep", bufs=1) as sp:
            xTm_s = sp.tile([C, B, N], F32, tag="xTm_s")
            xTmy_s = sp.tile([C, B, 512], F32, tag="xTmy_s")
            nc.sync.dma_start(xTm_s[:], xTm[:])
            nc.sync.dma_start(xTmy_s[:], xTmy[:])

            fmy = sp.tile([C, 512], F32)
            nc.vector.tensor_tensor(fmy[:], xTmy_s[:, 0], xTmy_s[:, 1], AL.add)
            nc.vector.tensor_tensor(fmy[:], fmy[:], xTmy_s[:, 2], AL.add)
            nc.vector.tensor_tensor(fmy[:], fmy[:], xTmy_s[:, 3], AL.add)

            fm = sp.tile([C, N], F32)
            nc.gpsimd.tensor_tensor(fm[:], xTm_s[:, 0], xTm_s[:, 1], AL.add)
            nc.gpsimd.tensor_tensor(fm[:], fm[:], xTm_s[:, 2], AL.add)
            nc.gpsimd.tensor_tensor(fm[:], fm[:], xTm_s[:, 3], AL.add)
            nc.gpsimd.tensor_scalar_mul(fm[:], fm[:], 0.25)

            fsq = sp.tile([C, N], F32, tag="fsq")
            nc.vector.tensor_tensor(fsq[:], fm[:], fm[:], AL.mult)
            onesC = sp.tile([C, 1], F32)
            nc.vector.memset(onesC[:], 1.0)
            nsq = sp.tile([1, N], F32)
            with tc.tile_pool(name="psP", bufs=2, space="PSUM") as psP:
                for ch in range(4):
                    ps = psP.tile([1, 512], F32, tag="ps1")
                    nc.tensor.matmul(ps[:], lhsT=onesC[:],
                                     rhs=fsq[:, ch * 512:(ch + 1) * 512],
                                     start=True, stop=True)
                    nc.scalar.activation(nsq[:, ch * 512:(ch + 1) * 512], ps[:], AF.Copy)
            nc.vector.tensor_scalar_max(nsq[:], nsq[:], 1e-24)
            nc.vector.reciprocal(nsq[:], nsq[:])
            nc.scalar.activation(nsq[:], nsq[:], AF.Sqrt)
            nc.sync.dma_start(ninv_d[:], nsq[:])
            ninv_rep = sp.tile([C, N], F32, tag="ninv_rep")
            nc.sync.dma_start(ninv_rep[:], ninv_d[0:1, :].to_broadcast([C, N]))
            fhat = sp.tile([C, N], F32, tag="fhat")
            nc.vector.tensor_tensor(fhat[:], fm[:], ninv_rep[:], AL.mult)

            # FM1 early: u1 = (x @ W1^T) at own rows (scale by d after RS)
            with tc.tile_pool(name="psU", bufs=2, space="PSUM") as psU:
                for t in range(4):
                    for b in range(B):
                        psy = psU.tile([P, F], F32, tag="psy")
                        nc.tensor.matmul(psy[:], lhsT=xTmy_s[:, b, t * P:(t + 1) * P],
                                         rhs=w1s[:], start=True, stop=True)
                        nc.scalar.activation(u1[:, t, b * F:(b + 1) * F], psy[:], AF.Copy)

            # ============ sim rows + top-k masks (own 4 tiles) ============
            with tc.tile_pool(name="topk", bufs=2) as tkp, \
                 tc.tile_pool(name="psS", bufs=2, space="PSUM") as psS:
                scr = sp.tile([P, 8], F32)
                inv8 = sp.tile([P, 8], F32)
                for t in range(4):
                    sim_sb = tkp.tile([P, N], F32, tag="simsb")
                    for ch in range(4):
                        ps = psS.tile([P, 512], F32, tag="ps512")
                        nc.tensor.matmul(ps[:], lhsT=fmy[:, t * P:(t + 1) * P],
                                         rhs=fhat[:, ch * 512:(ch + 1) * 512],
                                         start=True, stop=True)
                        nc.scalar.activation(sim_sb[:, ch * 512:(ch + 1) * 512], ps[:], AF.Copy)
                    work = tkp.tile([P, N], F32, tag="work")
                    src = sim_sb
                    for r in range(3):
                        nc.vector.max(out=scr[:], in_=src[:])
                        nc.vector.tensor_tensor(scr[:], scr[:], sm[:, r * 8:(r + 1) * 8], AL.mult)
                        nc.vector.tensor_scalar(inv8[:], sm[:, r * 8:(r + 1) * 8],
                                                -MINVAL, MINVAL, AL.mult, AL.add)
                        nc.vector.tensor_tensor(scr[:], scr[:], inv8[:], AL.add)
                        nc.vector.match_replace(out=work[:], in_to_replace=scr[:],
                                                in_values=src[:], imm_value=MINVAL)
                        src = work
                        if r == 0:
                            nc.vector.tensor_tensor(mk[0][:, t, :], work[:], sim_sb[:],
                                                    AL.not_equal)
                    nc.vector.tensor_tensor(mk[1][:, t, :], work[:], sim_sb[:], AL.not_equal)

                # scale masks: mk[q] *= 1/(k_q+1)  (so matmul gives w_q M^T M)
                mflat0 = mk[0][:].rearrange("p t n -> p (t n)")
                mflat1 = mk[1][:].rearrange("p t n -> p (t n)")
                nc.vector.tensor_scalar(mflat0, mflat0, invkA, None, AL.mult)
                nc.vector.tensor_scalar(mflat1, mflat1, invkB, None, AL.mult)

        if DEBUG:
            nc.sync.dma_start(dbg_mk[:].rearrange("p (q t n) -> q p t n", q=2, t=4)[0],
                              mk[0][:])
            nc.sync.dma_start(dbg_mk[:].rearrange("p (q t n) -> q p t n", q=2, t=4)[1],
                              mk[1][:])

        # ============ partial Sigma + colsums -> rsinS -> RS ============
        with tc.tile_pool(name="sig", bufs=2) as sgp:
            ones16 = sgp.tile([P, 1], F16, tag="ones16")
            nc.vector.memset(ones16[:], 1.0)
            # colsums (scaled): cs'_q = colsum(M_q)/(k_q+1)
            csf = sgp.tile([1, 2 * N], F16, tag="csf")
            with tc.tile_pool(name="psCS", bufs=2, space="PSUM") as psCS:
                for q in range(2):
                    for ch in range(4):
                        psc = psCS.tile([P, 512], F32, tag=f"cs{ch % 2}")
                        for t in range(4):
                            nc.tensor.matmul(psc[0:1, :], lhsT=ones16[:],
                                             rhs=mk[q][:, t, ch * 512:(ch + 1) * 512],
                                             start=(t == 0), stop=(t == 3))
                        nc.scalar.activation(
                            csf[:, q * N + ch * 512:q * N + (ch + 1) * 512],
                            psc[0:1, :], AF.Copy)
            nc.sync.dma_start(csflat_d[:], csf[:])
            # scatter cs' into per-dest regions (dram->dram, contiguous rows)
            csv = csflat_d[:].rearrange("a (q n) -> (a q) n", q=2)
            for d in range(4):
                nc.sync.dma_start(
                    cs_in[d:d + 1, :].rearrange("a (q x) -> (a q) x", q=2),
                    csv[:, d * 512:(d + 1) * 512])

            # partial Sigma chunks: psum[mt, ch] = sum_q sum_t mk_q[t, mt]^T mk_q[t, ch]
            with tc.tile_pool(name="psSG", bufs=1, space="PSUM") as psSG:
                for mt in range(NT):
                    for ch in range(4):
                        psg = psSG.tile([P, 512], F32, tag=f"sg{ch}")
                        for q in range(2):
                            for t in range(4):
                                nc.tensor.matmul(
                                    psg[:], lhsT=mk[q][:, t, mt * P:(mt + 1) * P],
                                    rhs=mk[q][:, t, ch * 512:(ch + 1) * 512],
                                    start=(q == 0 and t == 0), stop=(q == 1 and t == 3))
                        stg = sgp.tile([P, 512], F16, tag=f"stg{ch}")
                        nc.scalar.activation(stg[:], psg[:], AF.Copy)
                        nc.sync.dma_start(
                            sig_in[mt // 4, mt % 4, :, ch * 512:(ch + 1) * 512], stg[:])

            nc.gpsimd.collective_compute("ReduceScatter", AL.add, replica_groups=MODW,
                                         ins=[rsinS[:]], outs=[rsoutS[:]])

        # ============ post-RS: slabT, degrees, u1 variants ============
        with tc.tile_pool(name="post", bufs=1) as pq, \
             tc.tile_pool(name="psQ", bufs=2, space="PSUM") as psQ:
            nc.sync.dma_start(slabT[:], sig_out[:].rearrange("t p n -> p t n"))
            cst8 = pq.tile([8, P], F16, tag="cst8")
            nc.sync.dma_start(cst8[:],
                              cs_out[:].rearrange("a (k t p) -> (a k t) p", k=2, t=4, p=P))
            pst = psQ.tile([P, 8], F16, tag="pst")
            nc.tensor.transpose(pst[:], cst8[:], id128h[0:8, 0:8])
            cs_own = pq.tile([P, 8], F32, tag="cs_own")
            nc.scalar.activation(cs_own[:], pst[:], AF.Copy)
            dv = pq.tile([P, 4], F32, tag="dv")
            dvb = pq.tile([P, 4], F32, tag="dvb")
            nc.vector.tensor_scalar(dv[:], cs_own[:, 0:4], kA1, 1.0, AL.mult, AL.add)
            nc.vector.tensor_scalar(dvb[:], cs_own[:, 4:8], kB1, None, AL.mult)
            nc.vector.tensor_tensor(dv[:], dv[:], dvb[:], AL.add)
            nc.vector.reciprocal(d2_own[:], dv[:])
            nc.scalar.activation(d_own[:], d2_own[:], AF.Sqrt)

            if DEBUG:
                nc.sync.dma_start(dbg_slabT[:].rearrange("p (t n) -> p t n", t=4), slabT[:])
                nc.sync.dma_start(dbg_cs[:], cs_own[:])
                dcat = pq.tile([P, 8], F32, tag="dcat")
                nc.vector.tensor_copy(dcat[:, 0:4], d_own[:])
                nc.vector.tensor_copy(dcat[:, 4:8], d2_own[:])
                nc.sync.dma_start(dbg_d[:], dcat[:])

            # u1 variants: u1d = d*u1; u1g[h] = f_h * u1d
            for t in range(4):
                nc.vector.tensor_scalar(u1d[:, t], u1[:, t], d_own[:, t:t + 1],
                                        None, AL.mult)
            for h, fl in enumerate((f0a, f1a)):
                nc.gpsimd.tensor_scalar(u1g[h][:].rearrange("p t f -> p (t f)"),
                                        u1d[:].rearrange("p t f -> p (t f)"),
                                        fl, None, AL.mult)

        # ============ AGG1 -> RS1 ============
        with tc.tile_pool(name="agg1", bufs=2) as a1p, \
             tc.tile_pool(name="psA1", bufs=1, space="PSUM") as psA1:
            for g in range(GT):
                ps = psA1.tile([P, BF], F32, tag=f"a{g % 4}")
                for t in range(4):
                    nc.tensor.matmul(ps[:], lhsT=slabT[:, t, (g % 16) * P:(g % 16 + 1) * P],
                                     rhs=u1g[g // 16][:, t], start=(t == 0), stop=False)
                nc.tensor.matmul(ps[:], lhsT=jdg[:, g, :], rhs=u1d[:, g % 4],
                                 start=False, stop=True)
                stg = a1p.tile([P, BF], F16, tag=f"s{g % 4}")
                nc.scalar.activation(stg[:], ps[:], AF.Copy)
                nc.sync.dma_start(rsin1[g // 4, g % 4], stg[:])
            if DEBUG:
                nc.sync.dma_start(dbg_u1d[:].rearrange("p (t f) -> p t f", t=4), u1d[:])
                nc.sync.dma_start(dbg_rsin1[:],
                                  rsin1[:].rearrange("d t p f -> (d t) (p f)"))
            nc.gpsimd.collective_compute("ReduceScatter", AL.add, replica_groups=ALLW,
                                         ins=[rsin1[:]], outs=[rsout1[:]])

        # ============ x1 = relu(d*y1); x1T; u2 = d2*(x1@W2^T) ============
        with tc.tile_pool(name="mid", bufs=1) as mp, \
             tc.tile_pool(name="psM", bufs=1, space="PSUM") as psM:
            y1 = mp.tile([P, 4, BF], F16, tag="y1")
            nc.sync.dma_start(y1[:], rsout1[:].rearrange("t p f -> p t f"))
            if DEBUG:
                nc.sync.dma_start(dbg_y1[:].rearrange("p (t f) -> p t f", t=4), y1[:])
            x1 = mp.tile([P, 4, BF], F16, tag="x1")
            for t in range(4):
                nc.scalar.activation(x1[:, t], y1[:, t], AF.Relu, scale=d_own[:, t:t + 1])
            x1T = mp.tile([P, 16, P], F16, tag="x1T")
            for t in range(4):
                for b in range(B):
                    pst2 = psM.tile([P, P], F16, tag=f"tr{(t * B + b) % 4}")
                    nc.tensor.transpose(pst2[:], x1[:, t, b * F:(b + 1) * F], id128h[:])
                    nc.scalar.activation(x1T[:, t * 4 + b, :], pst2[:], AF.Copy)
            for t in range(4):
                for b in range(B):
                    psy = psM.tile([P, F], F32, tag=f"fm{(t * B + b) % 4}")
                    nc.tensor.matmul(psy[:], lhsT=x1T[:, t * 4 + b, :], rhs=w2s[:],
                                     start=True, stop=True)
                    nc.scalar.activation(u2d[:, t, b * F:(b + 1) * F], psy[:], AF.Copy,
                                         scale=d_own[:, t:t + 1])
            for h, fl in enumerate((f0a, f1a)):
                nc.gpsimd.tensor_scalar(u2g[h][:].rearrange("p t f -> p (t f)"),
                                        u2d[:].rearrange("p t f -> p (t f)"),
                                        fl, None, AL.mult)

        # ============ AGG2 -> RS2 -> out ============
        with tc.tile_pool(name="agg2", bufs=2) as a2p, \
             tc.tile_pool(name="psA2", bufs=1, space="PSUM") as psA2:
            for g in range(GT):
                ps = psA2.tile([P, BF], F32, tag=f"a{g % 4}")
                for t in range(4):
                    nc.tensor.matmul(ps[:], lhsT=slabT[:, t, (g % 16) * P:(g % 16 + 1) * P],
                                     rhs=u2g[g // 16][:, t], start=(t == 0), stop=False)
                nc.tensor.matmul(ps[:], lhsT=jdg[:, g, :], rhs=u2d[:, g % 4],
                                 start=False, stop=True)
                stg = a2p.tile([P, BF], F16, tag=f"s{g % 4}")
                nc.scalar.activation(stg[:], ps[:], AF.Copy)
                nc.sync.dma_start(rsin2[g // 4, g % 4], stg[:])
            nc.gpsimd.collective_compute("ReduceScatter", AL.add, replica_groups=ALLW,
                                         ins=[rsin2[:]], outs=[rsout2[:]])

        with tc.tile_pool(name="fin", bufs=1) as fp:
            y2 = fp.tile([P, 4, BF], F16, tag="y2")
            nc.sync.dma_start(y2[:], rsout2[:].rearrange("t p f -> p t f"))
            outsb = fp.tile([P, 4, B, F], F32, tag="outsb")
            for t in range(4):
                nc.scalar.activation(outsb[:, t].rearrange("p b f -> p (b f)"),
                                     y2[:, t], AF.Relu, scale=d_own[:, t:t + 1])
            for t in range(4):
                nc.sync.dma_start(
                    out_z[:, t * P:(t + 1) * P, :].rearrange("b p f -> p b f"),
                    outsb[:, t])

    nc.compile()
    return nc


def _make_inputs(feat_mod1, feat_mod2, W1, W2):
    f1 = np.ascontiguousarray(np.asarray(feat_mod1), np.float32)
    f2 = np.ascontiguousarray(np.asarray(feat_mod2), np.float32)
    xT1 = np.ascontiguousarray(f1.transpose(2, 0, 1))
    xT2 = np.ascontiguousarray(f2.transpose(2, 0, 1))
    w1t = np.ascontiguousarray(np.asarray(W1, np.float32).T)
    w2t = np.ascontiguousarray(np.asarray(W2, np.float32).T.astype(np.float16))

    KS = {0: (7, 19), 1: (5, 13)}  # k+1 per modality
    in_maps = []
    for c in range(8):
        m, s = c // 4, c % 4
        xTm = xT1 if m == 0 else xT2
        xTmy = np.ascontiguousarray(xTm[:, :, s * 512:(s + 1) * 512])
        kA, kB = KS[m]
        slotm = np.zeros((P, 24), np.float32)
        slotm[:, 0:kA] = 1.0
        slotm[:, 8:16] = 1.0
        rem = kB - kA - 8
        if rem > 0:
            slotm[:, 16:16 + rem] = 1.0
        f0 = 1.0 if m == 0 else 0.0
        ccv = np.zeros((P, 8), np.float32)
        ccv[:, 0] = 1.0 / kA
        ccv[:, 1] = 1.0 / kB
        ccv[:, 2] = float(kA)
        ccv[:, 3] = float(kB)
        ccv[:, 4] = f0
        ccv[:, 5] = 1.0 - f0
        jd = np.zeros((P, GT, P), np.float16)
        for g in range(GT):
            if s * 4 <= (g % 16) < (s + 1) * 4:
                jd[np.arange(P), g, np.arange(P)] = 0.25
        in_maps.append({
            "xTm": xTm, "xTmy": xTmy, "w1t": w1t, "w2t": w2t,
            "slotmask": slotm, "cconst": ccv,
            "jdiag": np.ascontiguousarray(jd.reshape(P, GT * P)),
        })
    return in_maps


def kernel(feat_mod1, feat_mod2, W1, W2):
    global _CACHED_NC, LAST_EXEC_TIME_NS, LAST_RESULTS
    if _CACHED_NC is None:
        _CACHED_NC = build_nc()
    in_maps = _make_inputs(feat_mod1, feat_mod2, W1, W2)
    res = run_bass_kernel_spmd(_CACHED_NC, in_maps, list(range(8)))
    LAST_RESULTS = res
    LAST_EXEC_TIME_NS = getattr(res, "exec_time_ns", None)
    outs = [res.results[c]["out_z"] for c in range(8)]
    out1 = np.concatenate(outs[0:4], axis=1)
    out2 = np.concatenate(outs[4:8], axis=1)
    return out1, out2


# revision 12
# speedup vs baseline: 2.3045x; 1.1355x over previous
"""Trainium2 Bass kernel for nn_DualModalHyperGraph (dual-modal hypergraph conv).

Self-contained: builds one SPMD Bass/Tile program for 8 NeuronCores, shards
inputs on the host, runs via run_bass_kernel_spmd, reassembles the output.

Math (equivalent to the reference):
  sim_m = col-normalized(mean_B feat_m) gram matrix     (per modality m)
  M_mk[n, r] = 1 iff r in top-(k+1) of row n            (two k per modality)
  Sigma_m = sum_k M_mk^T M_mk / (k+1)^2                 ([2048, 2048])
  S = blkdiag(Sigma_1, Sigma_2) + 1/4 [[I,I],[I,I]]
  A = D^-1/2 S D^-1/2,  D = diag(rowsum of H)
  x1 = relu(A (x @ W1^T)); x2 = relu(A (x1 @ W2^T))

Distribution (core c = 4*m + s: modality m, 512-row slab s):
  - Each core computes sim rows + top-k masks for its own 512 rows only.
  - Partial Sigma_m (contraction over own rows) is computed locally from
    scaled fp16 masks and summed via a per-modality ReduceScatter that also
    carries partial column-sum (degree) vectors; each core receives
    Sigma_m[own 512 rows, all 2048 modality cols] (slabT) plus its degrees.
  - Both hgconv layers are computed as partial contributions over own rows
    (y += S[own, :]^T u[own]) summed by 8-core ReduceScatters; the J = 1/4
    inter-modality block is applied via host-gated 0.25-identity matmuls.
  - SPMD divergence is data-driven only (host flag/diagonal tensors).
"""

import numpy as np

import concourse.bass as bass
import concourse.bacc as bacc
import concourse.mybir as mybir
import concourse.tile as tile
from concourse.bass_utils import run_bass_kernel_spmd
from concourse.masks import make_identity

P = 128
B = 4
N = 2048          # nodes per modality
C = 64
F = 128
NT = 16           # 128-row tiles per modality
GT = 32           # global row tiles
BF = B * F        # 512
MINVAL = -3.0e38
F16 = mybir.dt.float16
F32 = mybir.dt.float32
FP8 = mybir.dt.float8e4
AL = mybir.AluOpType
AF = mybir.ActivationFunctionType

SIGD = 4 * P * N          # per-dest Sigma elems in the RS-Sigma buffer
CSD = 2 * 512             # per-dest colsum elems
DD = SIGD + CSD

DEBUG = False
_CACHED_NC = None
LAST_EXEC_TIME_NS = None
LAST_RESULTS = None


def build_nc():
    nc = bacc.Bacc("TRN2", target_bir_lowering=False, debug=False, num_devices=8)

    xTm = nc.dram_tensor("xTm", [C, B, N], F32, kind="ExternalInput")
    xTmy = nc.dram_tensor("xTmy", [C, B, 512], F32, kind="ExternalInput")
    w1t = nc.dram_tensor("w1t", [C, F], F32, kind="ExternalInput")
    w2t = nc.dram_tensor("w2t", [F, F], F16, kind="ExternalInput")
    slotmask = nc.dram_tensor("slotmask", [P, 24], F32, kind="ExternalInput")
    cconst = nc.dram_tensor("cconst", [P, 8], F32, kind="ExternalInput")
    jdiag = nc.dram_tensor("jdiag", [P, GT * P], F16, kind="ExternalInput")
    out_z = nc.dram_tensor("out_z", [B, 512, F], F32, kind="ExternalOutput")
    if DEBUG:
        dbg_mk = nc.dram_tensor("dbg_mk", [P, 8 * N], FP8, kind="ExternalOutput")
        dbg_slabT = nc.dram_tensor("dbg_slabT", [P, 4 * N], F16, kind="ExternalOutput")
        dbg_cs = nc.dram_tensor("dbg_cs", [P, 8], F32, kind="ExternalOutput")
        dbg_d = nc.dram_tensor("dbg_d", [P, 8], F32, kind="ExternalOutput")
        dbg_u1d = nc.dram_tensor("dbg_u1d", [P, 4 * BF], F16, kind="ExternalOutput")
        dbg_y1 = nc.dram_tensor("dbg_y1", [P, 4 * BF], F16, kind="ExternalOutput")
        dbg_rsin1 = nc.dram_tensor("dbg_rsin1", [GT, P * BF], F16, kind="ExternalOutput")

    ninv_d = nc.dram_tensor("ninv_d", [1, N], F32)
    csflat_d = nc.dram_tensor("csflat_d", [1, 2 * N], F16)
    rsinS = nc.dram_tensor("rsinS", [4, DD], F16)
    rsoutS = nc.dram_tensor("rsoutS", [1, DD], F16)
    rsin1 = nc.dram_tensor("rsin1", [8, 4, P, BF], F16)
    rsout1 = nc.dram_tensor("rsout1", [4, P, BF], F16)
    rsin2 = nc.dram_tensor("rsin2", [8, 4, P, BF], F16)
    rsout2 = nc.dram_tensor("rsout2", [4, P, BF], F16)

    ALLW = [list(range(8))]
    MODW = [[0, 1, 2, 3], [4, 5, 6, 7]]

    # dram views
    sig_in = rsinS[:, 0:SIGD].rearrange("d (t p n) -> d t p n", t=4, p=P)
    cs_in = rsinS[:, SIGD:DD]                       # [4, 1024]
    sig_out = rsoutS[:, 0:SIGD].rearrange("a (t p n) -> (a t) p n", t=4, p=P)
    cs_out = rsoutS[:, SIGD:DD]                     # [1, 1024]

    with tile.TileContext(nc) as tc:
      with tc.tile_pool(name="persist", bufs=1) as pp:
        cc = pp.tile([P, 8], F32)
        sm = pp.tile([P, 24], F32)
        jdg = pp.tile([P, GT, P], F16)
        w1s = pp.tile([C, F], F32)
        w2s = pp.tile([F, F], F16)
        nc.sync.dma_start(cc[:], cconst[:])
        nc.sync.dma_start(sm[:], slotmask[:])
        nc.sync.dma_start(jdg[:], jdiag[:].rearrange("p (g q) -> p g q", g=GT))
        nc.sync.dma_start(w1s[:], w1t[:])
        nc.sync.dma_start(w2s[:], w2t[:])
        sc_a, sc_b = cc[:, 0:1], cc[:, 1:2]
        sc_c1, sc_c2 = cc[:, 2:3], cc[:, 3:4]
        f0a, f1a = cc[:, 4:5], cc[:, 5:6]
        wBa, invba = cc[:, 6:7], cc[:, 7:8]

        id128h = pp.tile([P, P], F16)
        make_identity(nc, id128h[:])

        # fp8 top-k mask buffers (12 rows: mB x4 | mA*a x4 | mA*c1 x4, and the
        # rhs twin with b/c2 scales) so one DoubleRow psum chain yields
        # wB*SigmaB + wA*SigmaA after the wB post-scale (a*b+c1*c2 ~= wA/wB)
        ml = pp.tile([P, 12, N], FP8, name="ml")
        mr = pp.tile([P, 12, N], FP8, name="mr")
        # degrees at own rows
        d_own = pp.tile([P, 4], F32)
        d2_own = pp.tile([P, 4], F32)
        # slabT = Sigma_m[own 512 rows, all N cols] (fp16, post-RS)
        slabT = pp.tile([P, 4, N], F16)
        # layer activations
        u1 = pp.tile([P, 4, BF], F16)      # x@W1 at own rows (unscaled)
        u1d = pp.tile([P, 4, BF], F16)     # d * u1
        u1g = [pp.tile([P, 4, BF], F16, name=f"u1g{h}") for h in range(2)]
        u2d = pp.tile([P, 4, BF], F16)
        u2g = [pp.tile([P, 4, BF], F16, name=f"u2g{h}") for h in range(2)]

        # ================= mean features, fhat, sim prep =================
        with tc.tile_pool(name="prep", bufs=1) as sp:
            xTmy_s = sp.tile([C, B, 512], F32, tag="xTmy_s")
            nc.sync.dma_start(xTmy_s[:], xTmy[:])

            fmy = sp.tile([C, 512], F32)
            nc.vector.tensor_tensor(fmy[:], xTmy_s[:, 0], xTmy_s[:, 1], AL.add)
            nc.vector.tensor_tensor(fmy[:], fmy[:], xTmy_s[:, 2], AL.add)
            nc.vector.tensor_tensor(fmy[:], fmy[:], xTmy_s[:, 3], AL.add)

            fm = sp.tile([C, N], F32)
            fhat = sp.tile([C, N], F32, tag="fhat")
            with tc.tile_pool(name="xload", bufs=1) as xl:
                xTm_s = xl.tile([C, B, N], F32, tag="xTm_s")
                nc.sync.dma_start(xTm_s[:], xTm[:])
                nc.gpsimd.tensor_tensor(fm[:], xTm_s[:, 0], xTm_s[:, 1], AL.add)
                nc.gpsimd.tensor_tensor(fm[:], fm[:], xTm_s[:, 2], AL.add)
                nc.gpsimd.tensor_tensor(fm[:], fm[:], xTm_s[:, 3], AL.add)
                nc.gpsimd.tensor_scalar_mul(fm[:], fm[:], 0.25)

                fsq = xl.tile([C, N], F32, tag="fsq")
                nc.vector.tensor_tensor(fsq[:], fm[:], fm[:], AL.mult)
                onesC = sp.tile([C, 1], F32)
                nc.vector.memset(onesC[:], 1.0)
                nsq = xl.tile([1, N], F32, tag="nsq")
                with tc.tile_pool(name="psP", bufs=2, space="PSUM") as psP:
                    for ch in range(4):
                        ps = psP.tile([1, 512], F32, tag="ps1")
                        nc.tensor.matmul(ps[:], lhsT=onesC[:],
                                         rhs=fsq[:, ch * 512:(ch + 1) * 512],
                                         start=True, stop=True)
                        nc.scalar.activation(nsq[:, ch * 512:(ch + 1) * 512], ps[:], AF.Copy)
                nc.vector.tensor_scalar_max(nsq[:], nsq[:], 1e-24)
                nc.vector.reciprocal(nsq[:], nsq[:])
                nc.scalar.activation(nsq[:], nsq[:], AF.Sqrt)
                nc.sync.dma_start(ninv_d[:], nsq[:])
                ninv_rep = xl.tile([C, N], F32, tag="ninv_rep")
                nc.sync.dma_start(ninv_rep[:], ninv_d[0:1, :].to_broadcast([C, N]))
                nc.vector.tensor_tensor(fhat[:], fm[:], ninv_rep[:], AL.mult)

            # FM1 early: u1 = (x @ W1^T) at own rows (scale by d after RS)
            with tc.tile_pool(name="psU", bufs=2, space="PSUM") as psU:
                for t in range(4):
                    for b in range(B):
                        psy = psU.tile([P, F], F32, tag="psy")
                        nc.tensor.matmul(psy[:], lhsT=xTmy_s[:, b, t * P:(t + 1) * P],
                                         rhs=w1s[:], start=True, stop=True)
                        nc.scalar.activation(u1[:, t, b * F:(b + 1) * F], psy[:], AF.Copy)

            # ============ sim rows + top-k masks (own 4 tiles) ============
            with tc.tile_pool(name="topk", bufs=2) as tkp, \
                 tc.tile_pool(name="psS", bufs=2, space="PSUM") as psS:
                scr = sp.tile([P, 8], F32)
                inv8 = sp.tile([P, 8], F32)
                for t in range(4):
                    sim_sb = tkp.tile([P, N], F32, tag="simsb")
                    for ch in range(4):
                        ps = psS.tile([P, 512], F32, tag="ps512")
                        nc.tensor.matmul(ps[:], lhsT=fmy[:, t * P:(t + 1) * P],
                                         rhs=fhat[:, ch * 512:(ch + 1) * 512],
                                         start=True, stop=True)
                        nc.scalar.activation(sim_sb[:, ch * 512:(ch + 1) * 512], ps[:], AF.Copy)
                    work = tkp.tile([P, N], F32, tag="work")
                    src = sim_sb
                    for r in range(3):
                        nc.vector.max(out=scr[:], in_=src[:])
                        nc.vector.tensor_tensor(scr[:], scr[:], sm[:, r * 8:(r + 1) * 8], AL.mult)
                        nc.vector.tensor_scalar(inv8[:], sm[:, r * 8:(r + 1) * 8],
                                                -MINVAL, MINVAL, AL.mult, AL.add)
                        nc.vector.tensor_tensor(scr[:], scr[:], inv8[:], AL.add)
                        nc.vector.match_replace(out=work[:], in_to_replace=scr[:],
                                                in_values=src[:], imm_value=MINVAL)
                        src = work
                        if r == 0:
                            tA = tkp.tile([P, N], FP8, tag="tA")
                            nc.vector.tensor_tensor(tA[:], work[:], sim_sb[:],
                                                    AL.not_equal)
                    nc.vector.tensor_tensor(ml[:, t, :], work[:], sim_sb[:], AL.not_equal)
                    nc.gpsimd.tensor_copy(mr[:, t, :], ml[:, t, :])
                    nc.gpsimd.tensor_scalar(ml[:, 4 + t, :], tA[:], sc_a, None, AL.mult)
                    nc.gpsimd.tensor_scalar(mr[:, 4 + t, :], tA[:], sc_b, None, AL.mult)
                    nc.gpsimd.tensor_scalar(ml[:, 8 + t, :], tA[:], sc_c1, None, AL.mult)
                    nc.gpsimd.tensor_scalar(mr[:, 8 + t, :], tA[:], sc_c2, None, AL.mult)

        if DEBUG:
            nc.sync.dma_start(dbg_mk[:].rearrange("p (q t n) -> q p t n", q=2, t=4)[0],
                              ml[:, 4:8, :])
            nc.sync.dma_start(dbg_mk[:].rearrange("p (q t n) -> q p t n", q=2, t=4)[1],
                              ml[:, 0:4, :])

        # ============ partial Sigma + colsums -> rsinS -> RS ============
        with tc.tile_pool(name="sig", bufs=2) as sgp:
            ones8 = sgp.tile([P, 1], FP8, tag="ones8")
            nc.vector.memset(ones8[:], 1.0)
            # colsums: q=0 -> b*csA (mr rows 4..8), q=1 -> csB (mr rows 0..4)
            csf = sgp.tile([1, 2 * N], F16, tag="csf")
            with tc.tile_pool(name="psCS", bufs=2, space="PSUM") as psCS:
                for q, base in ((0, 4), (1, 0)):
                    for ch in range(4):
                        psc = psCS.tile([P, 512], F32, tag=f"cs{ch % 2}")
                        for t in range(4):
                            nc.tensor.matmul(psc[0:1, :], lhsT=ones8[:],
                                             rhs=mr[:, base + t,
                                                    ch * 512:(ch + 1) * 512],
                                             start=(t == 0), stop=(t == 3))
                        nc.scalar.activation(
                            csf[:, q * N + ch * 512:q * N + (ch + 1) * 512],
                            psc[0:1, :], AF.Copy)
            nc.sync.dma_start(csflat_d[:], csf[:])
            # scatter cs' into per-dest regions (dram->dram, contiguous rows)
            csv = csflat_d[:].rearrange("a (q n) -> (a q) n", q=2)
            for d in range(4):
                nc.sync.dma_start(
                    cs_in[d:d + 1, :].rearrange("a (q x) -> (a q) x", q=2),
                    csv[:, d * 512:(d + 1) * 512])

            # partial Sigma chunks via fp8 DoubleRow (6 pair-matmuls each)
            with tc.tile_pool(name="psSG", bufs=1, space="PSUM") as psSG:
                for mt in range(NT):
                    stg = sgp.tile([P, 4 * 512], F16, tag=f"stg{mt % 2}")
                    for ch in range(4):
                        psg = psSG.tile([P, 512], F32, tag=f"sg{ch}")
                        for j in range(6):
                            nc.tensor.matmul(
                                psg[:], lhsT=ml[:, 2 * j:2 * j + 2, mt * P:(mt + 1) * P],
                                rhs=mr[:, 2 * j:2 * j + 2, ch * 512:(ch + 1) * 512],
                                start=(j == 0), stop=(j == 5),
                                perf_mode=mybir.MatmulPerfMode.DoubleRow)
                        nc.scalar.activation(stg[:, ch * 512:(ch + 1) * 512], psg[:],
                                             AF.Copy, scale=wBa)
                    nc.sync.dma_start(sig_in[mt // 4, mt % 4, :, :], stg[:])

            nc.gpsimd.collective_compute("ReduceScatter", AL.add, replica_groups=MODW,
                                         ins=[rsinS[:]], outs=[rsoutS[:]])

        # ============ post-RS: slabT, degrees, u1 variants ============
        with tc.tile_pool(name="post", bufs=1) as pq, \
             tc.tile_pool(name="psQ", bufs=2, space="PSUM") as psQ:
            nc.sync.dma_start(slabT[:], sig_out[:].rearrange("t p n -> p t n"))
            cst8 = pq.tile([8, P], F16, tag="cst8")
            nc.sync.dma_start(cst8[:],
                              cs_out[:].rearrange("a (k t p) -> (a k t) p", k=2, t=4, p=P))
            pst = psQ.tile([P, 8], F16, tag="pst")
            nc.tensor.transpose(pst[:], cst8[:], id128h[0:8, 0:8])
            cs_own = pq.tile([P, 8], F32, tag="cs_own")
            nc.scalar.activation(cs_own[:], pst[:], AF.Copy)
            dv = pq.tile([P, 4], F32, tag="dv")
            nc.vector.tensor_scalar(dv[:], cs_own[:, 0:4], invba, 1.0, AL.mult, AL.add)
            nc.vector.tensor_tensor(dv[:], dv[:], cs_own[:, 4:8], AL.add)
            nc.vector.reciprocal(d2_own[:], dv[:])
            nc.scalar.activation(d_own[:], d2_own[:], AF.Sqrt)

            if DEBUG:
                nc.sync.dma_start(dbg_slabT[:].rearrange("p (t n) -> p t n", t=4), slabT[:])
                nc.sync.dma_start(dbg_cs[:], cs_own[:])
                dcat = pq.tile([P, 8], F32, tag="dcat")
                nc.vector.tensor_copy(dcat[:, 0:4], d_own[:])
                nc.vector.tensor_copy(dcat[:, 4:8], d2_own[:])
                nc.sync.dma_start(dbg_d[:], dcat[:])

            # u1 variants: u1d = d*u1; u1g[h] = f_h * u1d
            for t in range(4):
                nc.vector.tensor_scalar(u1d[:, t], u1[:, t], d_own[:, t:t + 1],
                                        None, AL.mult)
            for h, fl in enumerate((f0a, f1a)):
                nc.gpsimd.tensor_scalar(u1g[h][:].rearrange("p t f -> p (t f)"),
                                        u1d[:].rearrange("p t f -> p (t f)"),
                                        fl, None, AL.mult)

        # ============ AGG1 -> RS1 ============
        with tc.tile_pool(name="agg1", bufs=2) as a1p, \
             tc.tile_pool(name="psA1", bufs=1, space="PSUM") as psA1:
            for blk in range(4):
                pss = [psA1.tile([P, BF], F32, tag=f"a{i}", name=f"ps1_{i}") for i in range(8)]
                for t in range(4):
                    for gg in range(4):
                        for half in range(2):
                            nc.tensor.matmul(
                                pss[gg * 2 + half],
                                lhsT=slabT[:, t, (blk * 4 + gg) * P:(blk * 4 + gg + 1) * P],
                                rhs=u1g[half][:, t], start=(t == 0), stop=False)
                for gg in range(4):
                    for half in range(2):
                        g = 16 * half + blk * 4 + gg
                        nc.tensor.matmul(pss[gg * 2 + half], lhsT=jdg[:, g, :],
                                         rhs=u1d[:, gg], start=False, stop=True)
                stages = [a1p.tile([P, 4 * BF], F16, tag=f"st{h}", name=f"st1_{h}") for h in range(2)]
                for gg in range(4):
                    for half in range(2):
                        nc.scalar.activation(stages[half][:, gg * BF:(gg + 1) * BF],
                                             pss[gg * 2 + half], AF.Copy)
                for half in range(2):
                    nc.sync.dma_start(
                        rsin1[4 * half + blk].rearrange("t p f -> p t f"),
                        stages[half][:].rearrange("p (t f) -> p t f", t=4))
            if DEBUG:
                nc.sync.dma_start(dbg_u1d[:].rearrange("p (t f) -> p t f", t=4), u1d[:])
                nc.sync.dma_start(dbg_rsin1[:],
                                  rsin1[:].rearrange("d t p f -> (d t) (p f)"))
            nc.gpsimd.collective_compute("ReduceScatter", AL.add, replica_groups=ALLW,
                                         ins=[rsin1[:]], outs=[rsout1[:]])

        # ============ x1 = relu(d*y1); x1T; u2 = d2*(x1@W2^T) ============
        with tc.tile_pool(name="mid", bufs=1) as mp, \
             tc.tile_pool(name="psM", bufs=1, space="PSUM") as psM:
            y1 = mp.tile([P, 4, BF], F16, tag="y1")
            nc.sync.dma_start(y1[:], rsout1[:].rearrange("t p f -> p t f"))
            if DEBUG:
                nc.sync.dma_start(dbg_y1[:].rearrange("p (t f) -> p t f", t=4), y1[:])
            x1 = mp.tile([P, 4, BF], F16, tag="x1")
            for t in range(4):
                nc.scalar.activation(x1[:, t], y1[:, t], AF.Relu, scale=d_own[:, t:t + 1])
            x1T = mp.tile([P, 16, P], F16, tag="x1T")
            for t in range(4):
                for b in range(B):
                    pst2 = psM.tile([P, P], F16, tag=f"tr{(t * B + b) % 4}")
                    nc.tensor.transpose(pst2[:], x1[:, t, b * F:(b + 1) * F], id128h[:])
                    nc.scalar.activation(x1T[:, t * 4 + b, :], pst2[:], AF.Copy)
            for t in range(4):
                for b in range(B):
                    psy = psM.tile([P, F], F32, tag=f"fm{(t * B + b) % 4}")
                    nc.tensor.matmul(psy[:], lhsT=x1T[:, t * 4 + b, :], rhs=w2s[:],
                                     start=True, stop=True)
                    nc.scalar.activation(u2d[:, t, b * F:(b + 1) * F], psy[:], AF.Copy,
                                         scale=d_own[:, t:t + 1])
            for h, fl in enumerate((f0a, f1a)):
                nc.gpsimd.tensor_scalar(u2g[h][:].rearrange("p t f -> p (t f)"),
                                        u2d[:].rearrange("p t f -> p (t f)"),
                                        fl, None, AL.mult)

        # ============ AGG2 -> RS2 -> out ============
        with tc.tile_pool(name="agg2", bufs=2) as a2p, \
             tc.tile_pool(name="psA2", bufs=1, space="PSUM") as psA2:
            for blk in range(4):
                pss = [psA2.tile([P, BF], F32, tag=f"a{i}", name=f"ps2_{i}") for i in range(8)]
                for t in range(4):
                    for gg in range(4):
                        for half in range(2):
                            nc.tensor.matmul(
                                pss[gg * 2 + half],
                                lhsT=slabT[:, t, (blk * 4 + gg) * P:(blk * 4 + gg + 1) * P],
                                rhs=u2g[half][:, t], start=(t == 0), stop=False)
                for gg in range(4):
                    for half in range(2):
                        g = 16 * half + blk * 4 + gg
                        nc.tensor.matmul(pss[gg * 2 + half], lhsT=jdg[:, g, :],
                                         rhs=u2d[:, gg], start=False, stop=True)
                stages = [a2p.tile([P, 4 * BF], F16, tag=f"st{h}", name=f"st2_{h}") for h in range(2)]
                for gg in range(4):
                    for half in range(2):
                        nc.scalar.activation(stages[half][:, gg * BF:(gg + 1) * BF],
                                             pss[gg * 2 + half], AF.Copy)
                for half in range(2):
                    nc.sync.dma_start(
                        rsin2[4 * half + blk].rearrange("t p f -> p t f"),
                        stages[half][:].rearrange("p (t f) -> p t f", t=4))
            nc.gpsimd.collective_compute("ReduceScatter", AL.add, replica_groups=ALLW,
                                         ins=[rsin2[:]], outs=[rsout2[:]])

        with tc.tile_pool(name="fin", bufs=1) as fp:
            y2 = fp.tile([P, 4, BF], F16, tag="y2")
            nc.sync.dma_start(y2[:], rsout2[:].rearrange("t p f -> p t f"))
            outsb = fp.tile([P, 4, B, F], F32, tag="outsb")
            for t in range(4):
                nc.scalar.activation(outsb[:, t].rearrange("p b f -> p (b f)"),
                                     y2[:, t], AF.Relu, scale=d_own[:, t:t + 1])
            for t in range(4):
                nc.sync.dma_start(
                    out_z[:, t * P:(t + 1) * P, :].rearrange("b p f -> p b f"),
                    outsb[:, t])

    nc.compile()
    return nc


def _make_inputs(feat_mod1, feat_mod2, W1, W2):
    f1 = np.ascontiguousarray(np.asarray(feat_mod1), np.float32)
    f2 = np.ascontiguousarray(np.asarray(feat_mod2), np.float32)
    xT1 = np.ascontiguousarray(f1.transpose(2, 0, 1))
    xT2 = np.ascontiguousarray(f2.transpose(2, 0, 1))
    w1t = np.ascontiguousarray(np.asarray(W1, np.float32).T)
    w2t = np.ascontiguousarray(np.asarray(W2, np.float32).T.astype(np.float16))

    KS = {0: (7, 19), 1: (5, 13)}  # k+1 per modality
    in_maps = []
    for c in range(8):
        m, s = c // 4, c % 4
        xTm = xT1 if m == 0 else xT2
        xTmy = np.ascontiguousarray(xTm[:, :, s * 512:(s + 1) * 512])
        kA, kB = KS[m]
        slotm = np.zeros((P, 24), np.float32)
        slotm[:, 0:kA] = 1.0
        slotm[:, 8:16] = 1.0
        rem = kB - kA - 8
        if rem > 0:
            slotm[:, 16:16 + rem] = 1.0
        f0 = 1.0 if m == 0 else 0.0
        a, bsc, c1, c2 = ((3.25, 2.25, 0.25, 0.21875) if m == 0 else
                          (2.25, 3.0, 0.125, 0.078125))
        ccv = np.zeros((P, 8), np.float32)
        ccv[:, 0] = a
        ccv[:, 1] = bsc
        ccv[:, 2] = c1
        ccv[:, 3] = c2
        ccv[:, 4] = f0
        ccv[:, 5] = 1.0 - f0
        ccv[:, 6] = 1.0 / (kB * kB)
        ccv[:, 7] = 1.0 / bsc
        jd = np.zeros((P, GT, P), np.float16)
        for g in range(GT):
            if s * 4 <= (g % 16) < (s + 1) * 4:
                jd[np.arange(P), g, np.arange(P)] = 0.25
        in_maps.append({
            "xTm": xTm, "xTmy": xTmy, "w1t": w1t, "w2t": w2t,
            "slotmask": slotm, "cconst": ccv,
            "jdiag": np.ascontiguousarray(jd.reshape(P, GT * P)),
        })
    return in_maps


def kernel(feat_mod1, feat_mod2, W1, W2):
    global _CACHED_NC, LAST_EXEC_TIME_NS, LAST_RESULTS
    if _CACHED_NC is None:
        _CACHED_NC = build_nc()
    in_maps = _make_inputs(feat_mod1, feat_mod2, W1, W2)
    res = run_bass_kernel_spmd(_CACHED_NC, in_maps, list(range(8)))
    LAST_RESULTS = res
    LAST_EXEC_TIME_NS = getattr(res, "exec_time_ns", None)
    outs = [res.results[c]["out_z"] for c in range(8)]
    out1 = np.concatenate(outs[0:4], axis=1)
    out2 = np.concatenate(outs[4:8], axis=1)
    return out1, out2
